# revision 31
# baseline (speedup 1.0000x reference)
"""GCNII forward on 8 TRN2 NeuronCores (self-contained).

Strategy (balanced 1D row partitioning):
- nodes assigned to 160 (core,tile,slot) buckets round-robin by in-degree so
  every dst tile sees ~E/160 edges -> exactly 16 edge-chunks/tile; output
  rows un-permuted on the host.
- self-loop term handled locally on DVE from an SBUF-resident fp8 copy of
  the exchange values (self rows never gathered).
- exchange table in fp8e4 (dinv*h), AllGathered in 2 half-shard slices
  (first half triggered mid-tile-loop to overlap); double-buffered.
- per dst tile: 2 batched dma_gathers (<=1024 rows each) pull source rows
  into [128, 16, 1024] fp8 SBUF; scatter-add via one-hot fp8 DoubleRow
  matmuls; gathers round-robin over 4 SWDGE queues.
- z transposed for the layer GEMM via SBUF-source dma_gather(transpose=True)
  in bf16 (no PE transposes), cast to fp8 on DVE.
- layer GEMM in fp8 DoubleRow, identity-residual path kept in f32:
  h = relu((1-b)*z + (b/s)*(q8(z) @ q8(s*b*Wg))), s a power of two.
- h0 residual (0.1*h0) kept in SBUF as bf16; phase0 GEMM in bf16.
"""
import numpy as np
from ml_dtypes import float8_e4m3, bfloat16

import concourse.bass as bass
import concourse.mybir as mybir
import concourse.tile as tile
from concourse import bacc
from concourse.bass_utils import run_bass_kernel_spmd
from concourse.masks import make_identity

N, E = 20000, 320000
F_IN, H, C, L = 512, 1024, 64, 8
ALPHA, THETA = 0.1, 0.5
NCORES = 8
SHP = 2560                  # padded rows per core (20*128)
HALF = SHP // 2             # AllGather half-shard
V = NCORES * SHP            # table rows
P = 128
NT = SHP // P               # 20 dst tiles per core
NPB = 125                   # real nodes per bucket (20000/160)
KF = F_IN // P
KH = H // P

f32 = mybir.dt.float32
bf16 = mybir.dt.bfloat16
fp8 = mybir.dt.float8e4
i16 = mybir.dt.int16

BETAS = np.log(THETA / np.arange(1.0, L + 1.0) + 1.0).astype(np.float64)

_cache = {}


def _preprocess(x, edge_index, W1, b1, Wg, W2, b2):
    src = np.asarray(edge_index[0], dtype=np.int64)
    dst = np.asarray(edge_index[1], dtype=np.int64)
    deg = (np.bincount(dst, minlength=N) + 1).astype(np.float32)  # +self
    dinv = 1.0 / np.sqrt(deg)

    # balanced assignment: nodes sorted by in-degree, snake round-robin over
    # the 160 buckets; node -> (bucket, slot)
    nb = NCORES * NT
    order = np.argsort(-deg, kind="stable")
    bucket = np.zeros(N, dtype=np.int64)
    slot = np.zeros(N, dtype=np.int64)
    pos = np.arange(N)
    rnd = pos // nb
    off = pos % nb
    snake = np.where(rnd % 2 == 0, off, nb - 1 - off)
    bucket[order] = snake
    slot[order] = rnd
    assert slot.max() == NPB - 1
    core_of = bucket // NT
    tile_of = bucket % NT
    loc = tile_of * P + slot                    # row within core [0, 2560)
    # table row (2-half layout)
    trow = np.where(loc < HALF,
                    core_of * HALF + loc,
                    NCORES * HALF + core_of * HALF + (loc - HALF))

    # edges grouped by dst bucket
    gid = bucket[dst]
    eorder = np.argsort(gid, kind="stable")
    gid_s = gid[eorder]
    rows_s = trow[src[eorder]]
    dslot_s = slot[dst[eorder]]
    counts = np.bincount(gid_s, minlength=nb)
    starts = np.concatenate([[0], np.cumsum(counts)[:-1]])
    j = np.arange(len(gid_s)) - starts[gid_s]
    nch_t = np.ceil(counts.reshape(NCORES, NT) / P).astype(np.int64).max(0)
    nch_t += nch_t % 2                          # even for DoubleRow pairs
    TOTC = int(nch_t.sum())
    base = np.zeros(NT, dtype=np.int64)
    base[1:] = np.cumsum(nch_t)[:-1]

    c_idx = j // P
    p_idx = j % P
    idx16 = np.zeros((NCORES, 128, TOTC * 8), dtype=np.int16)
    S = np.zeros((NCORES, P, TOTC, P), dtype=float8_e4m3)
    core_s = gid_s // NT
    tl_s = gid_s % NT
    gcol = base[tl_s] + c_idx
    flat = gcol * P + p_idx
    icol = flat // 16
    irow = flat % 16
    for r in range(8):
        idx16[core_s, r * 16 + irow, icol] = rows_s.astype(np.int16)
    S[core_s, p_idx, gcol, dslot_s] = 1.0

    # per-core padded dinv columns [P, NT] (0 on pad slots)
    dpad = np.zeros((NCORES, NT, P), dtype=np.float32)
    dpad[core_of, tile_of, slot] = dinv
    dcols = dpad.transpose(0, 2, 1).copy()      # [c, P, NT]
    d09 = (0.9 * dcols).astype(np.float32)
    escale = np.zeros((NCORES, P, L * NT), dtype=np.float32)
    for l in range(L):
        escale[:, :, l * NT:(l + 1) * NT] = (1.0 - BETAS[l]) * dcols

    # x packed for lhsT: xp[c, t, p, k*128+m] = x[node(c,t,m), k*128+p]
    x = np.asarray(x, dtype=np.float32)
    xsh = np.zeros((NCORES, NT, P, F_IN), dtype=np.float32)
    xsh[core_of, tile_of, slot] = x
    xp = np.ascontiguousarray(
        xsh.reshape(NCORES, NT, P, KF, P).transpose(0, 1, 4, 3, 2)
    ).reshape(NCORES, NT, P, F_IN).astype(bfloat16)

    def pack_w(w, dt):  # [K, Nout] -> [P, K//P, Nout]
        K = w.shape[0]
        return np.ascontiguousarray(
            w.reshape(K // P, P, -1).transpose(1, 0, 2)).astype(dt)

    W1p = pack_w(np.asarray(W1, np.float32), bfloat16)
    W2p = pack_w(np.asarray(W2, np.float32), bfloat16)
    Wg = np.asarray(Wg, dtype=np.float64)
    scales = []
    Wqs = []
    for l in range(L):
        bw = BETAS[l] * Wg[l]
        s = 2.0 ** np.floor(np.log2(240.0 / np.abs(bw).max()))
        scales.append(float(s))
        Wqs.append(pack_w((s * bw).astype(np.float32), float8_e4m3))
    Wq = np.stack(Wqs)

    b1b = np.broadcast_to(np.asarray(b1, np.float32), (P, H)).copy()
    b2b = np.broadcast_to(np.asarray(b2, np.float32), (P, C)).copy()

    # identity idxs for the SBUF transpose-gather: flat i at (i%16, i//16),
    # replicated across the 8 gpsimd cores
    idT = np.tile(np.arange(P, dtype=np.int16).reshape(P // 16, 16).T,
                  (8, 1)).copy()

    # inverse permutation for the output: global row -> (core, loc)
    meta = {"nch": nch_t, "base": base, "TOTC": TOTC, "scales": scales,
            "core_of": core_of, "loc": loc}
    in_maps = []
    for c in range(NCORES):
        in_maps.append({
            "xp": xp[c],
            "W1p": W1p, "W2p": W2p, "Wq": Wq,
            "b1b": b1b, "b2b": b2b,
            "d09": d09[c], "dinvc": dcols[c].astype(np.float32),
            "escale": escale[c],
            "idx16": idx16[c], "Smat": S[c], "idT": idT,
        })
    return in_maps, meta


def _build(meta):
    TOTC = meta["TOTC"]
    nch0 = meta["nch"]
    base0 = meta["base"]
    scales = meta["scales"]
    NCHMAX = int(nch0.max())

    nc = bacc.Bacc("TRN2", target_bir_lowering=False, debug=False,
                   num_devices=NCORES, num_swdge_queues=4)
    t_xp = nc.dram_tensor("xp", [NT, P, F_IN], bf16, kind="ExternalInput")
    t_W1 = nc.dram_tensor("W1p", [P, KF, H], bf16, kind="ExternalInput")
    t_W2 = nc.dram_tensor("W2p", [P, KH, C], bf16, kind="ExternalInput")
    t_Wq = nc.dram_tensor("Wq", [L, P, KH, H], fp8, kind="ExternalInput")
    t_b1 = nc.dram_tensor("b1b", [P, H], f32, kind="ExternalInput")
    t_b2 = nc.dram_tensor("b2b", [P, C], f32, kind="ExternalInput")
    t_d09 = nc.dram_tensor("d09", [P, NT], f32, kind="ExternalInput")
    t_dinv = nc.dram_tensor("dinvc", [P, NT], f32, kind="ExternalInput")
    t_esc = nc.dram_tensor("escale", [P, L * NT], f32, kind="ExternalInput")
    t_idx = nc.dram_tensor("idx16", [128, TOTC * 8], i16, kind="ExternalInput")
    t_S = nc.dram_tensor("Smat", [P, TOTC, P], fp8, kind="ExternalInput")
    t_idT = nc.dram_tensor("idT", [128, 8], i16, kind="ExternalInput")
    t_out = nc.dram_tensor("out", [SHP, C], f32, kind="ExternalOutput")

    exch = nc.dram_tensor("exch", [SHP, H], fp8)
    tables = [nc.dram_tensor(f"tbl{i}", [V, H], fp8, addr_space="Shared")
              for i in range(2)]

    DR = mybir.MatmulPerfMode.DoubleRow
    ACT = mybir.ActivationFunctionType

    def allgather_half(dst_tbl, hf):
        nc.gpsimd.collective_compute(
            "AllGather", mybir.AluOpType.bypass,
            replica_groups=[list(range(NCORES))],
            ins=[exch.ap()[hf * HALF:(hf + 1) * HALF].opt()],
            outs=[dst_tbl.ap()[hf * NCORES * HALF:
                               (hf + 1) * NCORES * HALF].opt()])

    with tile.TileContext(nc) as tc:
        with (
            tc.tile_pool(name="const", bufs=1) as cp,
            tc.tile_pool(name="wpool", bufs=2) as wp,
            tc.tile_pool(name="xpool", bufs=2) as xp_,
            tc.tile_pool(name="gpool", bufs=2) as gp,
            tc.tile_pool(name="zpool", bufs=2) as zp,
            tc.tile_pool(name="ps_agg", bufs=2, space="PSUM") as pa,
            tc.tile_pool(name="ps_gemm", bufs=1, space="PSUM") as pg,
            tc.tile_pool(name="ps_tr", bufs=2, space="PSUM") as pt,
        ):
            ident = cp.tile([P, P], f32, tag="ident")
            make_identity(nc, ident[:])
            idx_sb = cp.tile([128, TOTC * 8], i16, tag="idx")
            nc.sync.dma_start(out=idx_sb[:], in_=t_idx[:])
            S_sb = cp.tile([P, TOTC, P], fp8, tag="S")
            nc.sync.dma_start(out=S_sb[:], in_=t_S[:])
            idT_sb = cp.tile([128, 8], i16, tag="idT")
            nc.sync.dma_start(out=idT_sb[:], in_=t_idT[:])
            d09_sb = cp.tile([P, NT], f32, tag="d09")
            nc.sync.dma_start(out=d09_sb[:], in_=t_d09[:])
            dinv_sb = cp.tile([P, NT], f32, tag="dinv")
            nc.sync.dma_start(out=dinv_sb[:], in_=t_dinv[:])
            esc_sb = cp.tile([P, L * NT], f32, tag="esc")
            nc.sync.dma_start(out=esc_sb[:], in_=t_esc[:])
            b1_sb = cp.tile([P, H], f32, tag="b1")
            nc.sync.dma_start(out=b1_sb[:], in_=t_b1[:])
            b2_sb = cp.tile([P, C], f32, tag="b2")
            nc.sync.dma_start(out=b2_sb[:], in_=t_b2[:])
            W1_sb = cp.tile([P, KF, H], bf16, tag="W1")
            nc.sync.dma_start(out=W1_sb[:], in_=t_W1[:])
            W2_sb = cp.tile([P, KH, C], bf16, tag="W2")
            nc.sync.dma_start(out=W2_sb[:], in_=t_W2[:])
            h0s_sb = cp.tile([P, NT, H], bf16, tag="h0s")
            e_sb = cp.tile([P, NT, H], fp8, tag="e")

            # ---- phase 0
            for t in range(NT):
                xt = xp_.tile([P, KF, P], bf16, tag="xt")
                nc.sync.dma_start(out=xt[:], in_=t_xp[t])
                ps = pg.tile([P, H], f32, space="PSUM", tag="gemm")
                for k in range(KF):
                    for nh in range(2):
                        nc.tensor.matmul(
                            out=ps[:, nh * 512:(nh + 1) * 512],
                            lhsT=xt[:, k, :],
                            rhs=W1_sb[:, k, nh * 512:(nh + 1) * 512],
                            start=(k == 0), stop=(k == KF - 1))
                nc.vector.tensor_add(out=ps[:], in0=ps[:], in1=b1_sb[:])
                nc.scalar.activation(out=h0s_sb[:, t, :], in_=ps[:],
                                     func=ACT.Relu, scale=0.1)
                nc.scalar.activation(out=e_sb[:, t, :], in_=ps[:],
                                     func=ACT.Relu,
                                     scale=dinv_sb[:, t:t + 1])
                nc.sync.dma_start(out=exch[t * P:(t + 1) * P, :],
                                  in_=e_sb[:, t, :])
                if t == NT - 7:
                    allgather_half(tables[0], 0)
            allgather_half(tables[0], 1)

            # ---- layers
            qctr = [0]
            for l in range(L):
                tbl = tables[l % 2]
                beta = float(BETAS[l])
                cprime = beta / ((1.0 - beta) * scales[l])
                Wq_sb = wp.tile([P, KH, H], fp8, tag="W")
                nc.sync.dma_start(out=Wq_sb[:], in_=t_Wq[l])
                def tile_front(t):
                    """gathers + agg matmuls + z extraction; returns z."""
                    nch_t = int(nch0[t])
                    b8 = int(base0[t]) * 8
                    bS = int(base0[t])
                    gparts = []
                    for cc0 in range(0, nch_t, 4):
                        w8 = min(4, nch_t - cc0)
                        gpart = gp.tile([P, 4, H], fp8,
                                        tag=f"g{cc0 // 4}", name="gpart")
                        nc.gpsimd.dma_gather(
                            out_ap=gpart[:, :w8, :], in_ap=tbl.ap(),
                            idxs_ap=idx_sb[:, b8 + cc0 * 8:
                                           b8 + (cc0 + w8) * 8],
                            num_idxs=w8 * P, num_idxs_reg=w8 * P,
                            elem_size=H, queue_num=qctr[0] % 4)
                        qctr[0] += 1
                        gparts.append(gpart)
                    # r = 0.9*dinv*e_self + 0.1*h0 (off the agg critical path)
                    vs = zp.tile([P, H], f32, tag="vs", bufs=1, name="vs")
                    nc.scalar.activation(out=vs[:], in_=e_sb[:, t, :],
                                         func=ACT.Copy,
                                         scale=d09_sb[:, t:t + 1])
                    r = zp.tile([P, H], f32, tag="r", name="r")
                    nc.vector.tensor_add(out=r[:], in0=vs[:],
                                         in1=h0s_sb[:, t, :])
                    agg = pa.tile([P, H], f32, space="PSUM", tag="agg",
                                  name="agg")
                    for kp in range(nch_t // 2):
                        gpart = gparts[(2 * kp) // 4]
                        cc = (2 * kp) % 4
                        for nh in range(2):
                            nc.tensor.matmul(
                                out=agg[:, nh * 512:(nh + 1) * 512],
                                lhsT=S_sb[:, bS + 2 * kp:bS + 2 * kp + 2, :],
                                rhs=gpart[:, cc:cc + 2,
                                          nh * 512:(nh + 1) * 512],
                                start=(kp == 0), stop=(kp == nch_t // 2 - 1),
                                perf_mode=DR)
                    # z = 0.9*dinv*agg + r
                    z = zp.tile([P, H], f32, tag="z", name="z")
                    nc.scalar.activation(out=z[:], in_=agg[:], func=ACT.Copy,
                                         scale=d09_sb[:, t:t + 1])
                    nc.vector.tensor_add(out=z[:], in0=z[:], in1=r[:])
                    return z

                def tile_back(t, z):
                    """transposes + GEMM + exchange for tile t."""
                    zqT = zp.tile([P, KH, P], fp8, tag="zqT", name="zqT")
                    for k in range(KH):
                        trp = pt.tile([P, P], f32, space="PSUM", tag="tr",
                                      name="trp")
                        nc.tensor.transpose(out=trp[:],
                                            in_=z[:, k * P:(k + 1) * P],
                                            identity=ident[:])
                        nc.vector.tensor_copy(out=zqT[:, k, :], in_=trp[:])
                    ps2 = pg.tile([P, H], f32, space="PSUM", tag="gemm",
                                  name="ps2")
                    for kp in range(KH // 2):
                        for nh in range(2):
                            nc.tensor.matmul(
                                out=ps2[:, nh * 512:(nh + 1) * 512],
                                lhsT=zqT[:, 2 * kp:2 * kp + 2, :],
                                rhs=Wq_sb[:, 2 * kp:2 * kp + 2,
                                          nh * 512:(nh + 1) * 512],
                                start=(kp == 0), stop=(kp == KH // 2 - 1),
                                perf_mode=DR)
                    w = zp.tile([P, H], f32, tag="w", name="w")
                    nc.scalar.activation(out=w[:], in_=ps2[:], func=ACT.Copy,
                                         scale=cprime)
                    nc.vector.tensor_add(out=w[:], in0=w[:], in1=z[:])
                    nc.scalar.activation(
                        out=e_sb[:, t, :], in_=w[:], func=ACT.Relu,
                        scale=esc_sb[:, l * NT + t:l * NT + t + 1])
                    nc.sync.dma_start(out=exch[t * P:(t + 1) * P, :],
                                      in_=e_sb[:, t, :])
                    if t == NT - 7:
                        allgather_half(tables[(l + 1) % 2], 0)

                def tile_back_final(t, z):
                    zqT = zp.tile([P, KH, P], fp8, tag="zqT", name="zqT")
                    for k in range(KH):
                        trp = pt.tile([P, P], f32, space="PSUM", tag="tr",
                                      name="trp")
                        nc.tensor.transpose(out=trp[:],
                                            in_=z[:, k * P:(k + 1) * P],
                                            identity=ident[:])
                        nc.vector.tensor_copy(out=zqT[:, k, :], in_=trp[:])
                    ps2 = pg.tile([P, H], f32, space="PSUM", tag="gemm",
                                  name="ps2")
                    for kp in range(KH // 2):
                        for nh in range(2):
                            nc.tensor.matmul(
                                out=ps2[:, nh * 512:(nh + 1) * 512],
                                lhsT=zqT[:, 2 * kp:2 * kp + 2, :],
                                rhs=Wq_sb[:, 2 * kp:2 * kp + 2,
                                          nh * 512:(nh + 1) * 512],
                                start=(kp == 0), stop=(kp == KH // 2 - 1),
                                perf_mode=DR)
                    w = zp.tile([P, H], f32, tag="w", name="w")
                    nc.scalar.activation(out=w[:], in_=ps2[:], func=ACT.Copy,
                                         scale=cprime)
                    nc.vector.tensor_add(out=w[:], in0=w[:], in1=z[:])
                    if True:
                        h8 = zp.tile([P, H], f32, tag="h8", bufs=1)
                        nc.scalar.activation(out=h8[:], in_=w[:],
                                             func=ACT.Relu,
                                             scale=1.0 - beta)
                        h8T = zp.tile([P, KH, P], bf16, tag="h8T", bufs=1)
                        for k in range(KH):
                            trp = pt.tile([P, P], f32, space="PSUM", tag="tr")
                            nc.tensor.transpose(out=trp[:],
                                                in_=h8[:, k * P:(k + 1) * P],
                                                identity=ident[:])
                            nc.vector.tensor_copy(out=h8T[:, k, :], in_=trp[:])
                        psl = pt.tile([P, P], f32, space="PSUM", tag="tr")
                        for k in range(KH):
                            nc.tensor.matmul(
                                out=psl[:, 0:C],
                                lhsT=h8T[:, k, :],
                                rhs=W2_sb[:, k, :],
                                start=(k == 0), stop=(k == KH - 1))
                        nc.vector.tensor_add(out=psl[:, 0:C], in0=psl[:, 0:C],
                                             in1=b2_sb[:])
                        mx = zp.tile([P, 1], f32, tag="mx")
                        nc.vector.tensor_reduce(out=mx[:], in_=psl[:, 0:C],
                                                axis=mybir.AxisListType.X,
                                                op=mybir.AluOpType.max)
                        nmx = zp.tile([P, 1], f32, tag="nmx")
                        nc.vector.tensor_scalar(
                            out=nmx[:], in0=mx[:], scalar1=-1.0, scalar2=None,
                            op0=mybir.AluOpType.mult)
                        esb = zp.tile([P, C], f32, tag="esb")
                        se = zp.tile([P, 1], f32, tag="se")
                        nc.scalar.activation(out=esb[:], in_=psl[:, 0:C],
                                             func=ACT.Exp,
                                             bias=nmx[:], accum_out=se[:])
                        lse = zp.tile([P, 1], f32, tag="lse")
                        nc.scalar.activation(out=lse[:], in_=se[:],
                                             func=ACT.Ln)
                        o_t = zp.tile([P, C], f32, tag="ot")
                        nc.vector.tensor_scalar(
                            out=o_t[:], in0=psl[:, 0:C], scalar1=mx[:],
                            scalar2=lse[:],
                            op0=mybir.AluOpType.subtract,
                            op1=mybir.AluOpType.subtract)
                        nc.sync.dma_start(out=t_out[t * P:(t + 1) * P, :],
                                          in_=o_t[:])

                back = tile_back if l < L - 1 else tile_back_final
                zprev = None
                for t in range(NT):
                    zc = tile_front(t)
                    if zprev is not None:
                        back(t - 1, zprev)
                    zprev = zc
                back(NT - 1, zprev)
                if l < L - 1:
                    allgather_half(tables[(l + 1) % 2], 1)
    nc.compile()
    return nc


def kernel(**inputs):
    in_maps, meta = _preprocess(
        inputs["x"], inputs["edge_index"], inputs["W1"], inputs["b1"],
        inputs["Wg"], inputs["W2"], inputs["b2"])
    key = ("nc", meta["TOTC"], tuple(meta["nch"]), tuple(meta["scales"]))
    if key not in _cache:
        _cache[key] = _build(meta)
    nc = _cache[key]
    res = run_bass_kernel_spmd(nc, in_maps, list(range(NCORES)))
    per_core = np.stack([res.results[c]["out"] for c in range(NCORES)])
    out = per_core[meta["core_of"], meta["loc"]]
    return out.astype(np.float32)


# revision 35
# speedup vs baseline: 2.4372x; 2.4372x over previous
"""GCNII forward on 8 TRN2 NeuronCores (self-contained).

Strategy (balanced 1D row partitioning):
- nodes assigned to 160 (core,tile,slot) buckets round-robin by in-degree so
  every dst tile sees ~E/160 edges -> exactly 16 edge-chunks/tile; output
  rows un-permuted on the host.
- self-loop term handled locally on DVE from an SBUF-resident fp8 copy of
  the exchange values (self rows never gathered).
- exchange table in fp8e4 (dinv*h), AllGathered in 2 half-shard slices
  (first half triggered mid-tile-loop to overlap); double-buffered.
- per dst tile: 2 batched dma_gathers (<=1024 rows each) pull source rows
  into [128, 16, 1024] fp8 SBUF; scatter-add via one-hot fp8 DoubleRow
  matmuls; gathers round-robin over 4 SWDGE queues.
- z transposed for the layer GEMM via SBUF-source dma_gather(transpose=True)
  in bf16 (no PE transposes), cast to fp8 on DVE.
- layer GEMM in fp8 DoubleRow, identity-residual path kept in f32:
  h = relu((1-b)*z + (b/s)*(q8(z) @ q8(s*b*Wg))), s a power of two.
- h0 residual (0.1*h0) kept in SBUF as bf16; phase0 GEMM in bf16.
"""
import numpy as np
from ml_dtypes import float8_e4m3, bfloat16

import concourse.bass as bass
import concourse.mybir as mybir
import concourse.tile as tile
from concourse import bacc
from concourse.bass_utils import run_bass_kernel_spmd
from concourse.masks import make_identity

N, E = 20000, 320000
F_IN, H, C, L = 512, 1024, 64, 8
ALPHA, THETA = 0.1, 0.5
NCORES = 8
SHP = 2560                  # padded rows per core (20*128)
SEC1 = 15 * 128             # AllGather slice 1 (tiles 0-14)
SEC2 = SHP - SEC1           # AllGather slice 2 (tiles 15-19)
V = NCORES * SHP            # table rows
P = 128
NT = SHP // P               # 20 dst tiles per core
NPB = 125                   # real nodes per bucket (20000/160)
KF = F_IN // P
KH = H // P

f32 = mybir.dt.float32
bf16 = mybir.dt.bfloat16
fp8 = mybir.dt.float8e4
i16 = mybir.dt.int16

BETAS = np.log(THETA / np.arange(1.0, L + 1.0) + 1.0).astype(np.float64)

_cache = {}


def _preprocess(x, edge_index, W1, b1, Wg, W2, b2):
    src = np.asarray(edge_index[0], dtype=np.int64)
    dst = np.asarray(edge_index[1], dtype=np.int64)
    deg = (np.bincount(dst, minlength=N) + 1).astype(np.float32)  # +self
    dinv = 1.0 / np.sqrt(deg)

    # balanced assignment: nodes sorted by in-degree, snake round-robin over
    # the 160 buckets; node -> (bucket, slot)
    nb = NCORES * NT
    order = np.argsort(-deg, kind="stable")
    bucket = np.zeros(N, dtype=np.int64)
    slot = np.zeros(N, dtype=np.int64)
    pos = np.arange(N)
    rnd = pos // nb
    off = pos % nb
    snake = np.where(rnd % 2 == 0, off, nb - 1 - off)
    bucket[order] = snake
    slot[order] = rnd
    assert slot.max() == NPB - 1
    core_of = bucket // NT
    tile_of = bucket % NT
    loc = tile_of * P + slot                    # row within core [0, 2560)
    # table row (2-section layout, 15/5 tile split)
    trow = np.where(loc < SEC1,
                    core_of * SEC1 + loc,
                    NCORES * SEC1 + core_of * SEC2 + (loc - SEC1))

    # edges grouped by dst bucket
    gid = bucket[dst]
    eorder = np.argsort(gid, kind="stable")
    gid_s = gid[eorder]
    rows_s = trow[src[eorder]]
    dslot_s = slot[dst[eorder]]
    counts = np.bincount(gid_s, minlength=nb)
    starts = np.concatenate([[0], np.cumsum(counts)[:-1]])
    j = np.arange(len(gid_s)) - starts[gid_s]
    nch_t = np.ceil(counts.reshape(NCORES, NT) / P).astype(np.int64).max(0)
    nch_t += nch_t % 2                          # even for DoubleRow pairs
    TOTC = int(nch_t.sum())
    base = np.zeros(NT, dtype=np.int64)
    base[1:] = np.cumsum(nch_t)[:-1]

    c_idx = j // P
    p_idx = j % P
    idx16 = np.zeros((NCORES, 128, TOTC * 8), dtype=np.int16)
    S = np.zeros((NCORES, P, TOTC, P), dtype=float8_e4m3)
    core_s = gid_s // NT
    tl_s = gid_s % NT
    gcol = base[tl_s] + c_idx
    flat = gcol * P + p_idx
    icol = flat // 16
    irow = flat % 16
    for r in range(8):
        idx16[core_s, r * 16 + irow, icol] = rows_s.astype(np.int16)
    S[core_s, p_idx, gcol, dslot_s] = 1.0

    # per-core padded dinv columns [P, NT] (0 on pad slots)
    dpad = np.zeros((NCORES, NT, P), dtype=np.float32)
    dpad[core_of, tile_of, slot] = dinv
    dcols = dpad.transpose(0, 2, 1).copy()      # [c, P, NT]
    d09 = (0.9 * dcols).astype(np.float32)
    escale = np.zeros((NCORES, P, L * NT), dtype=np.float32)
    for l in range(L):
        escale[:, :, l * NT:(l + 1) * NT] = (1.0 - BETAS[l]) * dcols

    # x packed for lhsT: xp[c, t, p, k*128+m] = x[node(c,t,m), k*128+p]
    x = np.asarray(x, dtype=np.float32)
    xsh = np.zeros((NCORES, NT, P, F_IN), dtype=np.float32)
    xsh[core_of, tile_of, slot] = x
    xp = np.ascontiguousarray(
        xsh.reshape(NCORES, NT, P, KF, P).transpose(0, 1, 4, 3, 2)
    ).reshape(NCORES, NT, P, F_IN).astype(bfloat16)

    def pack_w(w, dt):  # [K, Nout] -> [P, K//P, Nout]
        K = w.shape[0]
        return np.ascontiguousarray(
            w.reshape(K // P, P, -1).transpose(1, 0, 2)).astype(dt)

    W1p = pack_w(np.asarray(W1, np.float32), bfloat16)
    W2p = pack_w(np.asarray(W2, np.float32), bfloat16)
    Wg = np.asarray(Wg, dtype=np.float64)
    scales = []
    Wqs = []
    for l in range(L):
        bw = BETAS[l] * Wg[l]
        s = 2.0 ** np.floor(np.log2(240.0 / np.abs(bw).max()))
        scales.append(float(s))
        Wqs.append(pack_w((s * bw).astype(np.float32), float8_e4m3))
    Wq = np.stack(Wqs)

    b1b = np.broadcast_to(np.asarray(b1, np.float32), (P, H)).copy()
    b2b = np.broadcast_to(np.asarray(b2, np.float32), (P, C)).copy()

    # identity idxs for the SBUF transpose-gather: flat i at (i%16, i//16),
    # replicated across the 8 gpsimd cores
    idT = np.tile(np.arange(P, dtype=np.int16).reshape(P // 16, 16).T,
                  (8, 1)).copy()

    # inverse permutation for the output: global row -> (core, loc)
    meta = {"nch": nch_t, "base": base, "TOTC": TOTC, "scales": scales,
            "core_of": core_of, "loc": loc}
    in_maps = []
    for c in range(NCORES):
        in_maps.append({
            "xp": xp[c],
            "W1p": W1p, "W2p": W2p, "Wq": Wq,
            "b1b": b1b, "b2b": b2b,
            "d09": d09[c], "dinvc": dcols[c].astype(np.float32),
            "escale": escale[c],
            "idx16": idx16[c], "Smat": S[c], "idT": idT,
        })
    return in_maps, meta


def _build(meta):
    TOTC = meta["TOTC"]
    nch0 = meta["nch"]
    base0 = meta["base"]
    scales = meta["scales"]
    NCHMAX = int(nch0.max())

    nc = bacc.Bacc("TRN2", target_bir_lowering=False, debug=False,
                   num_devices=NCORES, num_swdge_queues=4)
    t_xp = nc.dram_tensor("xp", [NT, P, F_IN], bf16, kind="ExternalInput")
    t_W1 = nc.dram_tensor("W1p", [P, KF, H], bf16, kind="ExternalInput")
    t_W2 = nc.dram_tensor("W2p", [P, KH, C], bf16, kind="ExternalInput")
    t_Wq = nc.dram_tensor("Wq", [L, P, KH, H], fp8, kind="ExternalInput")
    t_b1 = nc.dram_tensor("b1b", [P, H], f32, kind="ExternalInput")
    t_b2 = nc.dram_tensor("b2b", [P, C], f32, kind="ExternalInput")
    t_d09 = nc.dram_tensor("d09", [P, NT], f32, kind="ExternalInput")
    t_dinv = nc.dram_tensor("dinvc", [P, NT], f32, kind="ExternalInput")
    t_esc = nc.dram_tensor("escale", [P, L * NT], f32, kind="ExternalInput")
    t_idx = nc.dram_tensor("idx16", [128, TOTC * 8], i16, kind="ExternalInput")
    t_S = nc.dram_tensor("Smat", [P, TOTC, P], fp8, kind="ExternalInput")
    t_idT = nc.dram_tensor("idT", [128, 8], i16, kind="ExternalInput")
    t_out = nc.dram_tensor("out", [SHP, C], f32, kind="ExternalOutput")

    exch = nc.dram_tensor("exch", [SHP, H], fp8)
    tables = [nc.dram_tensor(f"tbl{i}", [V, H], fp8, addr_space="Shared")
              for i in range(2)]

    DR = mybir.MatmulPerfMode.DoubleRow
    ACT = mybir.ActivationFunctionType

    def allgather_half(dst_tbl, hf):
        if hf == 0:
            ins = exch.ap()[0:SEC1]
            outs = dst_tbl.ap()[0:NCORES * SEC1]
        else:
            ins = exch.ap()[SEC1:SHP]
            outs = dst_tbl.ap()[NCORES * SEC1:V]
        nc.gpsimd.collective_compute(
            "AllGather", mybir.AluOpType.bypass,
            replica_groups=[list(range(NCORES))],
            ins=[ins.opt()], outs=[outs.opt()])

    with tile.TileContext(nc) as tc:
        with (
            tc.tile_pool(name="const", bufs=1) as cp,
            tc.tile_pool(name="wpool", bufs=2) as wp,
            tc.tile_pool(name="xpool", bufs=2) as xp_,
            tc.tile_pool(name="gpool", bufs=2) as gp,
            tc.tile_pool(name="zpool", bufs=2) as zp,
            tc.tile_pool(name="ps_agg", bufs=2, space="PSUM") as pa,
            tc.tile_pool(name="ps_gemm", bufs=1, space="PSUM") as pg,
            tc.tile_pool(name="ps_tr", bufs=2, space="PSUM") as pt,
        ):
            ident = cp.tile([P, P], f32, tag="ident")
            make_identity(nc, ident[:])
            idx_sb = cp.tile([128, TOTC * 8], i16, tag="idx")
            nc.sync.dma_start(out=idx_sb[:], in_=t_idx[:])
            S_sb = cp.tile([P, TOTC, P], fp8, tag="S")
            nc.sync.dma_start(out=S_sb[:], in_=t_S[:])
            idT_sb = cp.tile([128, 8], i16, tag="idT")
            nc.sync.dma_start(out=idT_sb[:], in_=t_idT[:])
            d09_sb = cp.tile([P, NT], f32, tag="d09")
            nc.sync.dma_start(out=d09_sb[:], in_=t_d09[:])
            dinv_sb = cp.tile([P, NT], f32, tag="dinv")
            nc.sync.dma_start(out=dinv_sb[:], in_=t_dinv[:])
            esc_sb = cp.tile([P, L * NT], f32, tag="esc")
            nc.sync.dma_start(out=esc_sb[:], in_=t_esc[:])
            b1_sb = cp.tile([P, H], f32, tag="b1")
            nc.sync.dma_start(out=b1_sb[:], in_=t_b1[:])
            b2_sb = cp.tile([P, C], f32, tag="b2")
            nc.sync.dma_start(out=b2_sb[:], in_=t_b2[:])
            W1_sb = cp.tile([P, KF, H], bf16, tag="W1")
            nc.sync.dma_start(out=W1_sb[:], in_=t_W1[:])
            W2_sb = cp.tile([P, KH, C], bf16, tag="W2")
            nc.sync.dma_start(out=W2_sb[:], in_=t_W2[:])
            h0s_sb = cp.tile([P, NT, H], bf16, tag="h0s")
            e_sb = cp.tile([P, NT, H], fp8, tag="e")

            # ---- phase 0
            for t in range(NT):
                xt = xp_.tile([P, KF, P], bf16, tag="xt")
                nc.sync.dma_start(out=xt[:], in_=t_xp[t])
                ps = pg.tile([P, H], f32, space="PSUM", tag="gemm")
                for k in range(KF):
                    for nh in range(2):
                        nc.tensor.matmul(
                            out=ps[:, nh * 512:(nh + 1) * 512],
                            lhsT=xt[:, k, :],
                            rhs=W1_sb[:, k, nh * 512:(nh + 1) * 512],
                            start=(k == 0), stop=(k == KF - 1))
                nc.vector.tensor_add(out=ps[:], in0=ps[:], in1=b1_sb[:])
                nc.scalar.activation(out=h0s_sb[:, t, :], in_=ps[:],
                                     func=ACT.Relu, scale=0.1)
                nc.scalar.activation(out=e_sb[:, t, :], in_=ps[:],
                                     func=ACT.Relu,
                                     scale=dinv_sb[:, t:t + 1])
                nc.sync.dma_start(out=exch[t * P:(t + 1) * P, :],
                                  in_=e_sb[:, t, :])
                if t == NT - 4:
                    allgather_half(tables[0], 0)
            allgather_half(tables[0], 1)

            # ---- layers
            qctr = [0]
            for l in range(L):
                tbl = tables[l % 2]
                beta = float(BETAS[l])
                cprime = beta / ((1.0 - beta) * scales[l])
                Wq_sb = wp.tile([P, KH, H], fp8, tag="W")
                nc.sync.dma_start(out=Wq_sb[:], in_=t_Wq[l])
                def tile_front(t):
                    """gathers + agg matmuls + z extraction; returns z."""
                    nch_t = int(nch0[t])
                    b8 = int(base0[t]) * 8
                    bS = int(base0[t])
                    gparts = []
                    for cc0 in range(0, nch_t, 4):
                        w8 = min(4, nch_t - cc0)
                        gpart = gp.tile([P, 4, H], fp8,
                                        tag=f"g{cc0 // 4}", name="gpart")
                        nc.gpsimd.dma_gather(
                            out_ap=gpart[:, :w8, :], in_ap=tbl.ap(),
                            idxs_ap=idx_sb[:, b8 + cc0 * 8:
                                           b8 + (cc0 + w8) * 8],
                            num_idxs=w8 * P, num_idxs_reg=w8 * P,
                            elem_size=H, queue_num=qctr[0] % 4)
                        qctr[0] += 1
                        gparts.append(gpart)
                    # r = 0.9*dinv*e_self + 0.1*h0 (off the agg critical path)
                    vs = zp.tile([P, H], f32, tag="vs", bufs=1, name="vs")
                    nc.scalar.activation(out=vs[:], in_=e_sb[:, t, :],
                                         func=ACT.Copy,
                                         scale=d09_sb[:, t:t + 1])
                    r = zp.tile([P, H], f32, tag="r", name="r")
                    nc.vector.tensor_add(out=r[:], in0=vs[:],
                                         in1=h0s_sb[:, t, :])
                    agg = pa.tile([P, H], f32, space="PSUM", tag="agg",
                                  name="agg")
                    for kp in range(nch_t // 2):
                        gpart = gparts[(2 * kp) // 4]
                        cc = (2 * kp) % 4
                        for nh in range(2):
                            nc.tensor.matmul(
                                out=agg[:, nh * 512:(nh + 1) * 512],
                                lhsT=S_sb[:, bS + 2 * kp:bS + 2 * kp + 2, :],
                                rhs=gpart[:, cc:cc + 2,
                                          nh * 512:(nh + 1) * 512],
                                start=(kp == 0), stop=(kp == nch_t // 2 - 1),
                                perf_mode=DR)
                    # z = 0.9*dinv*agg + r
                    z = zp.tile([P, H], f32, tag="z", name="z")
                    nc.scalar.activation(out=z[:], in_=agg[:], func=ACT.Copy,
                                         scale=d09_sb[:, t:t + 1])
                    nc.vector.tensor_add(out=z[:], in0=z[:], in1=r[:])
                    return z

                def tile_back(t, z):
                    """transposes + GEMM + exchange for tile t."""
                    zqT = zp.tile([P, KH, P], fp8, tag="zqT", name="zqT")
                    for k in range(KH):
                        trp = pt.tile([P, P], f32, space="PSUM", tag="tr",
                                      name="trp")
                        nc.tensor.transpose(out=trp[:],
                                            in_=z[:, k * P:(k + 1) * P],
                                            identity=ident[:])
                        nc.vector.tensor_copy(out=zqT[:, k, :], in_=trp[:])
                    ps2 = pg.tile([P, H], f32, space="PSUM", tag="gemm",
                                  name="ps2")
                    for kp in range(KH // 2):
                        for nh in range(2):
                            nc.tensor.matmul(
                                out=ps2[:, nh * 512:(nh + 1) * 512],
                                lhsT=zqT[:, 2 * kp:2 * kp + 2, :],
                                rhs=Wq_sb[:, 2 * kp:2 * kp + 2,
                                          nh * 512:(nh + 1) * 512],
                                start=(kp == 0), stop=(kp == KH // 2 - 1),
                                perf_mode=DR)
                    w = zp.tile([P, H], f32, tag="w", name="w")
                    nc.scalar.activation(out=w[:], in_=ps2[:], func=ACT.Copy,
                                         scale=cprime)
                    nc.vector.tensor_add(out=w[:], in0=w[:], in1=z[:])
                    nc.scalar.activation(
                        out=e_sb[:, t, :], in_=w[:], func=ACT.Relu,
                        scale=esc_sb[:, l * NT + t:l * NT + t + 1])
                    nc.sync.dma_start(out=exch[t * P:(t + 1) * P, :],
                                      in_=e_sb[:, t, :])
                    if t == NT - 4:
                        allgather_half(tables[(l + 1) % 2], 0)

                def tile_back_final(t, z):
                    zqT = zp.tile([P, KH, P], fp8, tag="zqT", name="zqT")
                    for k in range(KH):
                        trp = pt.tile([P, P], f32, space="PSUM", tag="tr",
                                      name="trp")
                        nc.tensor.transpose(out=trp[:],
                                            in_=z[:, k * P:(k + 1) * P],
                                            identity=ident[:])
                        nc.vector.tensor_copy(out=zqT[:, k, :], in_=trp[:])
                    ps2 = pg.tile([P, H], f32, space="PSUM", tag="gemm",
                                  name="ps2")
                    for kp in range(KH // 2):
                        for nh in range(2):
                            nc.tensor.matmul(
                                out=ps2[:, nh * 512:(nh + 1) * 512],
                                lhsT=zqT[:, 2 * kp:2 * kp + 2, :],
                                rhs=Wq_sb[:, 2 * kp:2 * kp + 2,
                                          nh * 512:(nh + 1) * 512],
                                start=(kp == 0), stop=(kp == KH // 2 - 1),
                                perf_mode=DR)
                    w = zp.tile([P, H], f32, tag="w", name="w")
                    nc.scalar.activation(out=w[:], in_=ps2[:], func=ACT.Copy,
                                         scale=cprime)
                    nc.vector.tensor_add(out=w[:], in0=w[:], in1=z[:])
                    if True:
                        h8 = zp.tile([P, H], f32, tag="h8", bufs=1)
                        nc.scalar.activation(out=h8[:], in_=w[:],
                                             func=ACT.Relu,
                                             scale=1.0 - beta)
                        h8T = zp.tile([P, KH, P], bf16, tag="h8T", bufs=1)
                        for k in range(KH):
                            trp = pt.tile([P, P], f32, space="PSUM", tag="tr")
                            nc.tensor.transpose(out=trp[:],
                                                in_=h8[:, k * P:(k + 1) * P],
                                                identity=ident[:])
                            nc.vector.tensor_copy(out=h8T[:, k, :], in_=trp[:])
                        psl = pt.tile([P, P], f32, space="PSUM", tag="tr")
                        for k in range(KH):
                            nc.tensor.matmul(
                                out=psl[:, 0:C],
                                lhsT=h8T[:, k, :],
                                rhs=W2_sb[:, k, :],
                                start=(k == 0), stop=(k == KH - 1))
                        nc.vector.tensor_add(out=psl[:, 0:C], in0=psl[:, 0:C],
                                             in1=b2_sb[:])
                        mx = zp.tile([P, 1], f32, tag="mx")
                        nc.vector.tensor_reduce(out=mx[:], in_=psl[:, 0:C],
                                                axis=mybir.AxisListType.X,
                                                op=mybir.AluOpType.max)
                        nmx = zp.tile([P, 1], f32, tag="nmx")
                        nc.vector.tensor_scalar(
                            out=nmx[:], in0=mx[:], scalar1=-1.0, scalar2=None,
                            op0=mybir.AluOpType.mult)
                        esb = zp.tile([P, C], f32, tag="esb")
                        se = zp.tile([P, 1], f32, tag="se")
                        nc.scalar.activation(out=esb[:], in_=psl[:, 0:C],
                                             func=ACT.Exp,
                                             bias=nmx[:], accum_out=se[:])
                        lse = zp.tile([P, 1], f32, tag="lse")
                        nc.scalar.activation(out=lse[:], in_=se[:],
                                             func=ACT.Ln)
                        o_t = zp.tile([P, C], f32, tag="ot")
                        nc.vector.tensor_scalar(
                            out=o_t[:], in0=psl[:, 0:C], scalar1=mx[:],
                            scalar2=lse[:],
                            op0=mybir.AluOpType.subtract,
                            op1=mybir.AluOpType.subtract)
                        nc.sync.dma_start(out=t_out[t * P:(t + 1) * P, :],
                                          in_=o_t[:])

                back = tile_back if l < L - 1 else tile_back_final
                zprev = None
                for t in range(NT):
                    zc = tile_front(t)
                    if zprev is not None:
                        back(t - 1, zprev)
                    zprev = zc
                back(NT - 1, zprev)
                if l < L - 1:
                    allgather_half(tables[(l + 1) % 2], 1)
    nc.compile()
    return nc


def kernel(**inputs):
    in_maps, meta = _preprocess(
        inputs["x"], inputs["edge_index"], inputs["W1"], inputs["b1"],
        inputs["Wg"], inputs["W2"], inputs["b2"])
    key = ("nc", meta["TOTC"], tuple(meta["nch"]), tuple(meta["scales"]))
    if key not in _cache:
        _cache[key] = _build(meta)
    nc = _cache[key]
    res = run_bass_kernel_spmd(nc, in_maps, list(range(NCORES)))
    per_core = np.stack([res.results[c]["out"] for c in range(NCORES)])
    out = per_core[meta["core_of"], meta["loc"]]
    return out.astype(np.float32)
